# revision 7
# baseline (speedup 1.0000x reference)
"""MoE layer (8 experts, top-2) for 8 Trainium2 NeuronCores.

Strategy: expert-parallel. The router (0.1% of FLOPs) runs on host and
decides the sharding: tokens are all-to-all'd by routed expert (host-side
gather, since kernel() holds the full inputs). Each core runs one expert's
dense MLP  y = scale * (gelu(x @ W1 + b1) @ W2 + b2)  over the tokens routed
to it, with fp32r (TF32-like full-rate) matmuls on the tensor engine.
Host scatter-adds the per-expert partial outputs back (the unshard).
"""

import os

import numpy as np

HIDDEN = 1024
FF = 2 * HIDDEN
NUM_EXPERTS = 8
TOP_K = 2
NCORES = 8

# Set by kernel() when MOE_TRACE=1: HW kernel execution time in ns.
LAST_EXEC_NS = None
LAST_RESULTS = None

_PROGRAM_CACHE = {}


def _round_up(v, m):
    return (v + m - 1) // m * m


def _build_program(C, blk):
    """Bass/Tile program for one expert MLP over C tokens (SPMD on 8 cores).

    Layouts (per core):
      xT  [HIDDEN, C] f32r  - gathered tokens, transposed
      w1  [HIDDEN, FF] f32r, b1 [FF] f32
      w2  [FF, HIDDEN] f32r, b2 [HIDDEN] f32
      scl [C] f32           - per-token combine weight
      yT  [HIDDEN, C] f32   - output, transposed

    Stage B: hT[f, t] = gelu(sum_h w1[h, f] * xT[h, t] + b1[f])  (PSUM acc over
    8 h-chunks; lhsT = w1 chunk [128h, 128f], rhs = xT chunk [128h, blk])
    Stage C: yT[o, t] = (sum_f w2[f, o] * hT[f, t] + b2[o]) * scl[t]
    """
    import concourse.bass as bass  # noqa: F401
    import concourse.mybir as mybir
    import concourse.tile as tile
    from concourse import bacc

    HC = HIDDEN // 128  # 8 h-chunks
    FC = FF // 128  # 16 f-chunks
    f32 = mybir.dt.float32
    f32r = mybir.dt.float32r

    nc = bacc.Bacc("TRN2", target_bir_lowering=False, debug=False,
                   num_devices=NCORES)
    xT = nc.dram_tensor("xT", [HIDDEN, C], f32r, kind="ExternalInput")
    w1 = nc.dram_tensor("w1", [HIDDEN, FF], f32r, kind="ExternalInput")
    b1 = nc.dram_tensor("b1", [FF], f32, kind="ExternalInput")
    w2 = nc.dram_tensor("w2", [FF, HIDDEN], f32r, kind="ExternalInput")
    b2 = nc.dram_tensor("b2", [HIDDEN], f32, kind="ExternalInput")
    scl = nc.dram_tensor("scl", [C], f32, kind="ExternalInput")
    yT = nc.dram_tensor("yT", [HIDDEN, C], f32, kind="ExternalOutput")

    blocks = []
    t0 = 0
    while t0 < C:
        b = min(blk, C - t0)
        blocks.append((t0, b))
        t0 += b

    Gelu = mybir.ActivationFunctionType.Gelu
    Ident = mybir.ActivationFunctionType.Identity

    with tile.TileContext(nc) as tc:
        with (
            tc.tile_pool(name="wts", bufs=1) as wts,
            tc.tile_pool(name="xin", bufs=2) as xin,
            tc.tile_pool(name="hmid", bufs=1) as hmid,
            tc.tile_pool(name="outs", bufs=3) as outs,
            tc.tile_pool(name="ps", bufs=4, space="PSUM") as ps,
        ):
            # --- resident weights/biases ---
            # Weights are loaded in chunks (separate tiles) so the first
            # matmuls only wait for the first slice, and the rest of the
            # ~17MB streams in underneath compute.
            W1G, W2G = 8, 8  # chunk counts for w1 (along f) and w2 (along fc)
            fper = FC // W1G  # f-chunks per w1 group
            cper = FC // W2G  # f-chunks per w2 group
            b1_sb = wts.tile([128, FC], f32)
            nc.sync.dma_start(
                out=b1_sb[:], in_=b1.ap().rearrange("(c p) -> p c", p=128))
            b2_sb = wts.tile([128, HC], f32)
            nc.sync.dma_start(
                out=b2_sb[:], in_=b2.ap().rearrange("(c p) -> p c", p=128))
            w1_g = []
            w2_g = []
            for g in range(W1G):
                t = wts.tile([128, HC, fper * 128], f32r, tag=f"w1g{g}")
                nc.sync.dma_start(
                    out=t[:],
                    in_=w1.ap().rearrange("(c p) f -> p c f", p=128)[
                        :, :, g * fper * 128:(g + 1) * fper * 128])
                w1_g.append(t)
            for g in range(W2G):
                t = wts.tile([128, cper, HIDDEN], f32r, tag=f"w2g{g}")
                nc.sync.dma_start(
                    out=t[:],
                    in_=w2.ap().rearrange("(c p) h -> p c h", p=128)[
                        :, g * cper:(g + 1) * cper, :])
                w2_g.append(t)

            def w1_lhsT(hc, fc):
                return w1_g[fc // fper][:, hc, (fc % fper) * 128:(fc % fper + 1) * 128]

            def w2_lhsT(fc, oc):
                return w2_g[fc // cper][:, fc % cper, oc * 128:(oc + 1) * 128]

            for t0, bs in blocks:
                x_sb = xin.tile([128, HC, bs], f32r, tag="x")
                nc.sync.dma_start(
                    out=x_sb[:],
                    in_=xT.ap().rearrange("(c p) t -> p c t", p=128)[:, :, t0:t0 + bs])
                s_sb = xin.tile([128, bs], f32, tag="s")
                nc.sync.dma_start(
                    out=s_sb[:], in_=scl.ap()[t0:t0 + bs].partition_broadcast(128))

                h_sb = hmid.tile([128, FC, blk], f32r, tag="h")
                for fc in range(FC):
                    ph = ps.tile([128, blk], f32, tag="ps")
                    for hc in range(HC):
                        nc.tensor.matmul(
                            ph[:, :bs],
                            w1_lhsT(hc, fc),
                            x_sb[:, hc, :],
                            start=(hc == 0), stop=(hc == HC - 1),
                        )
                    nc.scalar.activation(
                        out=h_sb[:, fc, :bs], in_=ph[:, :bs],
                        func=Gelu, bias=b1_sb[:, fc:fc + 1], scale=1.0)

                for oc in range(HC):
                    py = ps.tile([128, blk], f32, tag="ps")
                    for fc in range(FC):
                        nc.tensor.matmul(
                            py[:, :bs],
                            w2_lhsT(fc, oc),
                            h_sb[:, fc, :bs],
                            start=(fc == 0), stop=(fc == FC - 1),
                        )
                    o1 = outs.tile([128, blk], f32, tag="o1")
                    nc.scalar.activation(
                        out=o1[:, :bs], in_=py[:, :bs], func=Ident,
                        bias=b2_sb[:, oc:oc + 1], scale=1.0)
                    o2 = outs.tile([128, blk], f32, tag="o2")
                    nc.vector.tensor_mul(o2[:, :bs], o1[:, :bs], s_sb[:])
                    nc.sync.dma_start(
                        out=yT.ap().rearrange(
                            "(c p) t -> p c t", p=128)[:, oc, t0:t0 + bs],
                        in_=o2[:, :bs])

    nc.compile()
    return nc


def _route_host(x, Wr, br):
    """Replicate the reference router bit-exactly (jax on CPU)."""
    import jax
    import jax.numpy as jnp

    cpu = jax.devices("cpu")[0]
    xj = jax.device_put(x, cpu)
    Wrj = jax.device_put(Wr, cpu)
    brj = jax.device_put(br, cpu)
    with jax.default_device(cpu):
        logits = jnp.einsum("bsh,he->bse", xj, Wrj) + brj
        routing = jax.nn.softmax(logits, axis=-1)
        topw, topi = jax.lax.top_k(routing, TOP_K)
        topw = jax.nn.softmax(topw, axis=-1)
    return np.asarray(topw), np.asarray(topi)


def kernel(x, Wr, br, W1, b1, W2, b2):
    global LAST_EXEC_NS, LAST_RESULTS
    from concourse.bass_utils import run_bass_kernel_spmd

    x = np.ascontiguousarray(np.asarray(x, dtype=np.float32))
    Wr = np.asarray(Wr, dtype=np.float32)
    br = np.asarray(br, dtype=np.float32)
    W1 = np.ascontiguousarray(np.asarray(W1, dtype=np.float32))
    b1 = np.ascontiguousarray(np.asarray(b1, dtype=np.float32))
    W2 = np.ascontiguousarray(np.asarray(W2, dtype=np.float32))
    b2 = np.ascontiguousarray(np.asarray(b2, dtype=np.float32))

    B, S, H = x.shape
    ntok = B * S
    xf = x.reshape(ntok, H)

    topw, topi = _route_host(x, Wr, br)
    topw = topw.reshape(ntok, TOP_K)
    topi = topi.reshape(ntok, TOP_K)

    # per-expert token index lists + combine weights
    idx = []
    wgt = []
    for e in range(NUM_EXPERTS):
        mask = (topi == e)
        tok = np.nonzero(mask.any(axis=1))[0]
        w = (topw * mask).sum(axis=1)[tok].astype(np.float32)
        idx.append(tok)
        wgt.append(w)
    counts = np.array([len(t) for t in idx])

    blk = int(os.environ.get("MOE_BLK", "256"))
    C = max(_round_up(int(counts.max()), 128), blk)

    key = (C, blk)
    if key not in _PROGRAM_CACHE:
        _PROGRAM_CACHE[key] = _build_program(C, blk)
    nc = _PROGRAM_CACHE[key]

    in_maps = []
    for e in range(NUM_EXPERTS):
        xTe = np.zeros((H, C), dtype=np.float32)
        xTe[:, :counts[e]] = xf[idx[e]].T
        scle = np.zeros((C,), dtype=np.float32)
        scle[:counts[e]] = wgt[e]
        in_maps.append({
            "xT": xTe,
            "w1": np.ascontiguousarray(W1[e]),
            "b1": np.ascontiguousarray(b1[e]),
            "w2": np.ascontiguousarray(W2[e]),
            "b2": np.ascontiguousarray(b2[e]),
            "scl": scle,
        })

    trace = os.environ.get("MOE_TRACE", "0") == "1"
    res = run_bass_kernel_spmd(
        nc, in_maps, core_ids=list(range(NCORES)), trace=trace)
    LAST_EXEC_NS = res.exec_time_ns
    LAST_RESULTS = res

    out = np.zeros((ntok, H), dtype=np.float32)
    for e in range(NUM_EXPERTS):
        ye = res.results[e]["yT"][:, :counts[e]].T  # [cnt, H]
        out[idx[e]] += ye * 1.0
    return out.reshape(B, S, H)


# revision 8
# speedup vs baseline: 1.1524x; 1.1524x over previous
"""MoE layer (8 experts, top-2) for 8 Trainium2 NeuronCores.

Strategy: expert-parallel. The router (0.1% of FLOPs) runs on host and
decides the sharding: tokens are all-to-all'd by routed expert (host-side
gather, since kernel() holds the full inputs). Each core runs one expert's
dense MLP  y = scale * (gelu(x @ W1 + b1) @ W2 + b2)  over the tokens routed
to it, with fp32r (TF32-like full-rate) matmuls on the tensor engine.
Host scatter-adds the per-expert partial outputs back (the unshard).
"""

import os

import numpy as np

HIDDEN = 1024
FF = 2 * HIDDEN
NUM_EXPERTS = 8
TOP_K = 2
NCORES = 8

# Set by kernel() when MOE_TRACE=1: HW kernel execution time in ns.
LAST_EXEC_NS = None
LAST_RESULTS = None

_PROGRAM_CACHE = {}


def _round_up(v, m):
    return (v + m - 1) // m * m


def _build_program(C, blk):
    """Bass/Tile program for one expert MLP over C tokens (SPMD on 8 cores).

    Layouts (per core):
      xT  [HIDDEN, C] f32r  - gathered tokens, transposed
      w1  [HIDDEN, FF] f32r, b1 [FF] f32
      w2  [FF, HIDDEN] f32r, b2 [HIDDEN] f32
      scl [C] f32           - per-token combine weight
      yT  [HIDDEN, C] f32   - output, transposed

    Stage B: hT[f, t] = gelu(sum_h w1[h, f] * xT[h, t] + b1[f])  (PSUM acc over
    8 h-chunks; lhsT = w1 chunk [128h, 128f], rhs = xT chunk [128h, blk])
    Stage C: yT[o, t] = (sum_f w2[f, o] * hT[f, t] + b2[o]) * scl[t]
    """
    import concourse.bass as bass  # noqa: F401
    import concourse.mybir as mybir
    import concourse.tile as tile
    from concourse import bacc

    HC = HIDDEN // 128  # 8 h-chunks
    FC = FF // 128  # 16 f-chunks
    f32 = mybir.dt.float32
    f32r = mybir.dt.float32r

    nc = bacc.Bacc("TRN2", target_bir_lowering=False, debug=False,
                   num_devices=NCORES)
    xT = nc.dram_tensor("xT", [HIDDEN, C], f32r, kind="ExternalInput")
    w1 = nc.dram_tensor("w1", [HIDDEN, FF], f32r, kind="ExternalInput")
    b1 = nc.dram_tensor("b1", [FF], f32, kind="ExternalInput")
    w2 = nc.dram_tensor("w2", [FF, HIDDEN], f32r, kind="ExternalInput")
    b2 = nc.dram_tensor("b2", [HIDDEN], f32, kind="ExternalInput")
    scl = nc.dram_tensor("scl", [C], f32, kind="ExternalInput")
    yT = nc.dram_tensor("yT", [HIDDEN, C], f32, kind="ExternalOutput")

    blocks = []
    t0 = 0
    while t0 < C:
        b = min(blk, C - t0)
        blocks.append((t0, b))
        t0 += b

    Gelu = mybir.ActivationFunctionType.Gelu
    Ident = mybir.ActivationFunctionType.Identity

    with tile.TileContext(nc) as tc:
        with (
            tc.tile_pool(name="wts", bufs=1) as wts,
            tc.tile_pool(name="xin", bufs=2) as xin,
            tc.tile_pool(name="hmid", bufs=1) as hmid,
            tc.tile_pool(name="outs", bufs=3) as outs,
            tc.tile_pool(name="ps", bufs=4, space="PSUM") as ps,
        ):
            # --- resident weights/biases ---
            # Weights are loaded in chunks (separate tiles) so the first
            # matmuls only wait for the first slice, and the rest of the
            # ~17MB streams in underneath compute.
            W1G, W2G = 8, 8  # chunk counts for w1 (along f) and w2 (along fc)
            fper = FC // W1G  # f-chunks per w1 group
            cper = FC // W2G  # f-chunks per w2 group
            b1_sb = wts.tile([128, FC], f32)
            nc.gpsimd.dma_start(
                out=b1_sb[:], in_=b1.ap().rearrange("(c p) -> p c", p=128))
            b2_sb = wts.tile([128, HC], f32)
            nc.gpsimd.dma_start(
                out=b2_sb[:], in_=b2.ap().rearrange("(c p) -> p c", p=128))
            w1_g = []
            w2_g = []
            for g in range(W1G):
                t = wts.tile([128, HC, fper * 128], f32r, tag=f"w1g{g}")
                nc.gpsimd.dma_start(
                    out=t[:],
                    in_=w1.ap().rearrange("(c p) f -> p c f", p=128)[
                        :, :, g * fper * 128:(g + 1) * fper * 128])
                w1_g.append(t)
            for g in range(W2G):
                t = wts.tile([128, cper, HIDDEN], f32r, tag=f"w2g{g}")
                nc.gpsimd.dma_start(
                    out=t[:],
                    in_=w2.ap().rearrange("(c p) h -> p c h", p=128)[
                        :, g * cper:(g + 1) * cper, :])
                w2_g.append(t)

            def w1_lhsT(hc, fc):
                return w1_g[fc // fper][:, hc, (fc % fper) * 128:(fc % fper + 1) * 128]

            def w2_lhsT(fc, oc):
                return w2_g[fc // cper][:, fc % cper, oc * 128:(oc + 1) * 128]

            for t0, bs in blocks:
                x_sb = xin.tile([128, HC, bs], f32r, tag="x")
                nc.sync.dma_start(
                    out=x_sb[:],
                    in_=xT.ap().rearrange("(c p) t -> p c t", p=128)[:, :, t0:t0 + bs])
                s_sb = xin.tile([128, bs], f32, tag="s")
                nc.sync.dma_start(
                    out=s_sb[:], in_=scl.ap()[t0:t0 + bs].partition_broadcast(128))

                h_sb = hmid.tile([128, FC, blk], f32r, tag="h")
                for fc in range(FC):
                    ph = ps.tile([128, blk], f32, tag="ps")
                    for hc in range(HC):
                        nc.tensor.matmul(
                            ph[:, :bs],
                            w1_lhsT(hc, fc),
                            x_sb[:, hc, :],
                            start=(hc == 0), stop=(hc == HC - 1),
                        )
                    nc.scalar.activation(
                        out=h_sb[:, fc, :bs], in_=ph[:, :bs],
                        func=Gelu, bias=b1_sb[:, fc:fc + 1], scale=1.0)

                for oc in range(HC):
                    py = ps.tile([128, blk], f32, tag="ps")
                    for fc in range(FC):
                        nc.tensor.matmul(
                            py[:, :bs],
                            w2_lhsT(fc, oc),
                            h_sb[:, fc, :bs],
                            start=(fc == 0), stop=(fc == FC - 1),
                        )
                    o1 = outs.tile([128, blk], f32, tag="o1")
                    nc.scalar.activation(
                        out=o1[:, :bs], in_=py[:, :bs], func=Ident,
                        bias=b2_sb[:, oc:oc + 1], scale=1.0)
                    o2 = outs.tile([128, blk], f32, tag="o2")
                    nc.vector.tensor_mul(o2[:, :bs], o1[:, :bs], s_sb[:])
                    nc.scalar.dma_start(
                        out=yT.ap().rearrange(
                            "(c p) t -> p c t", p=128)[:, oc, t0:t0 + bs],
                        in_=o2[:, :bs])

    nc.compile()
    return nc


def _route_host(x, Wr, br):
    """Replicate the reference router bit-exactly (jax on CPU)."""
    import jax
    import jax.numpy as jnp

    cpu = jax.devices("cpu")[0]
    xj = jax.device_put(x, cpu)
    Wrj = jax.device_put(Wr, cpu)
    brj = jax.device_put(br, cpu)
    with jax.default_device(cpu):
        logits = jnp.einsum("bsh,he->bse", xj, Wrj) + brj
        routing = jax.nn.softmax(logits, axis=-1)
        topw, topi = jax.lax.top_k(routing, TOP_K)
        topw = jax.nn.softmax(topw, axis=-1)
    return np.asarray(topw), np.asarray(topi)


def kernel(x, Wr, br, W1, b1, W2, b2):
    global LAST_EXEC_NS, LAST_RESULTS
    from concourse.bass_utils import run_bass_kernel_spmd

    x = np.ascontiguousarray(np.asarray(x, dtype=np.float32))
    Wr = np.asarray(Wr, dtype=np.float32)
    br = np.asarray(br, dtype=np.float32)
    W1 = np.ascontiguousarray(np.asarray(W1, dtype=np.float32))
    b1 = np.ascontiguousarray(np.asarray(b1, dtype=np.float32))
    W2 = np.ascontiguousarray(np.asarray(W2, dtype=np.float32))
    b2 = np.ascontiguousarray(np.asarray(b2, dtype=np.float32))

    B, S, H = x.shape
    ntok = B * S
    xf = x.reshape(ntok, H)

    topw, topi = _route_host(x, Wr, br)
    topw = topw.reshape(ntok, TOP_K)
    topi = topi.reshape(ntok, TOP_K)

    # per-expert token index lists + combine weights
    idx = []
    wgt = []
    for e in range(NUM_EXPERTS):
        mask = (topi == e)
        tok = np.nonzero(mask.any(axis=1))[0]
        w = (topw * mask).sum(axis=1)[tok].astype(np.float32)
        idx.append(tok)
        wgt.append(w)
    counts = np.array([len(t) for t in idx])

    blk = int(os.environ.get("MOE_BLK", "256"))
    C = max(_round_up(int(counts.max()), 128), blk)

    key = (C, blk)
    if key not in _PROGRAM_CACHE:
        _PROGRAM_CACHE[key] = _build_program(C, blk)
    nc = _PROGRAM_CACHE[key]

    in_maps = []
    for e in range(NUM_EXPERTS):
        xTe = np.zeros((H, C), dtype=np.float32)
        xTe[:, :counts[e]] = xf[idx[e]].T
        scle = np.zeros((C,), dtype=np.float32)
        scle[:counts[e]] = wgt[e]
        in_maps.append({
            "xT": xTe,
            "w1": np.ascontiguousarray(W1[e]),
            "b1": np.ascontiguousarray(b1[e]),
            "w2": np.ascontiguousarray(W2[e]),
            "b2": np.ascontiguousarray(b2[e]),
            "scl": scle,
        })

    trace = os.environ.get("MOE_TRACE", "0") == "1"
    res = run_bass_kernel_spmd(
        nc, in_maps, core_ids=list(range(NCORES)), trace=trace)
    LAST_EXEC_NS = res.exec_time_ns
    LAST_RESULTS = res

    out = np.zeros((ntok, H), dtype=np.float32)
    for e in range(NUM_EXPERTS):
        ye = res.results[e]["yT"][:, :counts[e]].T  # [cnt, H]
        out[idx[e]] += ye * 1.0
    return out.reshape(B, S, H)


# revision 44
# speedup vs baseline: 1.2501x; 1.0848x over previous
"""MoE layer (8 experts, top-2) for 8 Trainium2 NeuronCores.

Strategy: expert-parallel. The router (0.1% of FLOPs) runs on host and
decides the sharding: tokens are all-to-all'd by routed expert (host-side
gather, since kernel() holds the full inputs). Each core runs one expert's
dense MLP  y = scale * (gelu(x @ W1 + b1) @ W2 + b2)  over the tokens routed
to it, with fp32r (TF32-like full-rate) matmuls on the tensor engine.
Host scatter-adds the per-expert partial outputs back (the unshard).
"""

import os

import numpy as np

HIDDEN = 1024
FF = 2 * HIDDEN
NUM_EXPERTS = 8
TOP_K = 2
NCORES = 8

# Set by kernel() when MOE_TRACE=1: HW kernel execution time in ns.
LAST_EXEC_NS = None
LAST_RESULTS = None

_PROGRAM_CACHE = {}


def _round_up(v, m):
    return (v + m - 1) // m * m


def _build_program(C, blk):
    """Bass/Tile program for one expert MLP over C tokens (SPMD on 8 cores).

    Layouts (per core):
      xT  [HIDDEN, C] f32r  - gathered tokens, transposed
      w1  [HIDDEN, FF] f32r, b1 [FF] f32
      w2  [FF, HIDDEN] f32r, b2 [HIDDEN] f32
      scl [C] f32           - per-token combine weight
      yT  [HIDDEN, C] f32   - output, transposed

    Stage B: hT[f, t] = gelu(sum_h w1[h, f] * xT[h, t] + b1[f])  (PSUM acc over
    8 h-chunks; lhsT = w1 chunk [128h, 128f], rhs = xT chunk [128h, blk])
    Stage C: yT[o, t] = (sum_f w2[f, o] * hT[f, t] + b2[o]) * scl[t]
    """
    import concourse.bass as bass  # noqa: F401
    import concourse.mybir as mybir
    import concourse.tile as tile
    from concourse import bacc

    HC = HIDDEN // 128  # 8 h-chunks
    FC = FF // 128  # 16 f-chunks
    f32 = mybir.dt.float32
    f32r = mybir.dt.float32r

    nc = bacc.Bacc("TRN2", target_bir_lowering=False, debug=False,
                   num_devices=NCORES)
    W1G, W2G = 16, 8  # dma chunk counts for w1 (along f) and w2 (along fc)
    FPER = FC // W1G  # f-chunks per w1 group
    CPER = FC // W2G  # f-chunks per w2 group
    # w1p/w2p are host-packed in SBUF tile layout (partition-major per
    # group) so every weight DMA reads >=4KB contiguous per partition.
    xT = nc.dram_tensor("xT", [HIDDEN, C], f32r, kind="ExternalInput")
    w1 = nc.dram_tensor(
        "w1p", [128, W1G, HC, FPER * 128], f32r, kind="ExternalInput")
    b1 = nc.dram_tensor("b1", [FF], f32, kind="ExternalInput")
    w2 = nc.dram_tensor(
        "w2p", [128, W2G, CPER, HIDDEN], f32r, kind="ExternalInput")
    b2 = nc.dram_tensor("b2", [HIDDEN], f32, kind="ExternalInput")
    scl = nc.dram_tensor("scl", [C], f32, kind="ExternalInput")
    yT = nc.dram_tensor("yT", [HIDDEN, C], f32, kind="ExternalOutput")

    # Token blocks: fp32r needs moving dim >= 256 for full PE rate, so split
    # the ragged tail into two >=256 pieces instead of leaving a small block.
    blocks = []
    t0 = 0
    rem = C
    while rem > 0:
        if rem >= blk + 256 or rem <= blk:
            b = min(blk, rem)
        else:
            b = rem - 256
        if b < 256 and rem > b:
            b = rem
        blocks.append((t0, b))
        t0 += b
        rem -= b

    Gelu = mybir.ActivationFunctionType.Gelu
    Ident = mybir.ActivationFunctionType.Identity

    with tile.TileContext(nc) as tc:
        with (
            tc.tile_pool(name="wts", bufs=1) as wts,
            tc.tile_pool(name="xin", bufs=10) as xin,
            tc.tile_pool(name="sin", bufs=2) as sin,
            tc.tile_pool(name="hmid", bufs=1) as hmid,
            tc.tile_pool(name="outs", bufs=2) as outs,
            tc.tile_pool(name="ps", bufs=4, space="PSUM") as ps,
        ):
            # --- resident weights/biases ---
            # Weights are loaded in chunks (separate tiles) so the first
            # matmuls only wait for the first slice, and the rest of the
            # ~17MB streams in underneath compute.
            fper, cper = FPER, CPER

            def in_ring():
                return nc.sync

            def emit_xs(t0, bs):
                xcs = []
                for hc in range(HC):
                    xc = xin.tile([128, blk], f32r, tag="x", name=f"x{hc}")
                    nc.sync.dma_start(
                        out=xc[:, :bs],
                        in_=xT.ap().rearrange(
                            "(c p) t -> p c t", p=128)[:, hc, t0:t0 + bs])
                    xcs.append(xc)
                s_sb = sin.tile([128, blk], f32, tag="s", name="s")
                nc.sync.dma_start(
                    out=s_sb[:, :bs],
                    in_=scl.ap()[t0:t0 + bs].partition_broadcast(128))
                return xcs, s_sb

            # Ring order: w1g0 first (the first matmul's weights), then
            # block 0's activations, then the rest of the weight stream
            # (w1 before w2 — consumption order), all HWDGE.
            w1_g = []
            w2_g = []
            for g in range(1):
                t = wts.tile([128, HC, fper * 128], f32r, tag=f"w1g{g}")
                in_ring().dma_start(out=t[:], in_=w1.ap()[:, g])
                w1_g.append(t)
            pre_x = {blocks[0]: emit_xs(*blocks[0])}
            for g in range(1, W1G):
                t = wts.tile([128, HC, fper * 128], f32r, tag=f"w1g{g}")
                in_ring().dma_start(out=t[:], in_=w1.ap()[:, g])
                w1_g.append(t)
            b1_sb = wts.tile([128, FC], f32)
            nc.scalar.dma_start(
                out=b1_sb[:], in_=b1.ap().rearrange("(c p) -> p c", p=128))
            b2_sb = wts.tile([128, HC], f32)
            nc.scalar.dma_start(
                out=b2_sb[:], in_=b2.ap().rearrange("(c p) -> p c", p=128))
            for g in range(W2G):
                t = wts.tile([128, cper, HIDDEN], f32r, tag=f"w2g{g}")
                in_ring().dma_start(out=t[:], in_=w2.ap()[:, g])
                w2_g.append(t)

            def w1_lhsT(hc, fc):
                return w1_g[fc // fper][:, hc, (fc % fper) * 128:(fc % fper + 1) * 128]

            def w2_lhsT(fc, oc):
                return w2_g[fc // cper][:, fc % cper, oc * 128:(oc + 1) * 128]

            for t0, bs in blocks:
                if (t0, bs) in pre_x:
                    x_sb, s_sb = pre_x[(t0, bs)]
                else:
                    x_sb, s_sb = emit_xs(t0, bs)

                h_sb = hmid.tile([128, FC, blk], f32r, tag="h")
                for fc in range(FC):
                    ph = ps.tile([128, blk], f32, tag="ps")
                    for hc in range(HC):
                        nc.tensor.matmul(
                            ph[:, :bs],
                            w1_lhsT(hc, fc),
                            x_sb[hc][:, :bs],
                            start=(hc == 0), stop=(hc == HC - 1),
                        )
                    nc.scalar.activation(
                        out=h_sb[:, fc, :bs], in_=ph[:, :bs],
                        func=Gelu, bias=b1_sb[:, fc:fc + 1], scale=1.0)

                for oc in range(HC):
                    py = ps.tile([128, blk], f32, tag="ps")
                    for fc in range(FC):
                        nc.tensor.matmul(
                            py[:, :bs],
                            w2_lhsT(fc, oc),
                            h_sb[:, fc, :bs],
                            start=(fc == 0), stop=(fc == FC - 1),
                        )
                    o1 = outs.tile([128, blk], f32, tag="o1")
                    nc.scalar.activation(
                        out=o1[:, :bs], in_=py[:, :bs], func=Ident,
                        bias=b2_sb[:, oc:oc + 1], scale=1.0)
                    nc.vector.tensor_mul(o1[:, :bs], o1[:, :bs], s_sb[:, :bs])
                    nc.scalar.dma_start(
                        out=yT.ap().rearrange(
                            "(c p) t -> p c t", p=128)[:, oc, t0:t0 + bs],
                        in_=o1[:, :bs])

    nc.compile()
    return nc


def _route_host(x, Wr, br):
    """Replicate the reference router bit-exactly (jax on CPU), with a
    numpy fallback (same math, same tie semantics) if jax-cpu is absent."""
    try:
        import jax
        import jax.numpy as jnp

        cpu = jax.devices("cpu")[0]
        xj = jax.device_put(x, cpu)
        Wrj = jax.device_put(Wr, cpu)
        brj = jax.device_put(br, cpu)
        with jax.default_device(cpu):
            logits = jnp.einsum("bsh,he->bse", xj, Wrj) + brj
            routing = jax.nn.softmax(logits, axis=-1)
            topw, topi = jax.lax.top_k(routing, TOP_K)
            topw = jax.nn.softmax(topw, axis=-1)
        return np.asarray(topw), np.asarray(topi)
    except Exception:
        lg = x.reshape(-1, x.shape[-1]).astype(np.float32) @ Wr + br
        m = lg.max(axis=-1, keepdims=True)
        p = np.exp(lg - m)
        p /= p.sum(axis=-1, keepdims=True)
        # top-k with lower-index-wins tie semantics (jax.lax.top_k)
        topi = np.argsort(-p, axis=-1, kind="stable")[:, :TOP_K]
        topv = np.take_along_axis(p, topi, axis=-1)
        e = np.exp(topv - topv.max(axis=-1, keepdims=True))
        topw = (e / e.sum(axis=-1, keepdims=True)).astype(np.float32)
        B, S = x.shape[0], x.shape[1]
        return (topw.reshape(B, S, TOP_K),
                topi.astype(np.int32).reshape(B, S, TOP_K))


def kernel(x, Wr, br, W1, b1, W2, b2):
    global LAST_EXEC_NS, LAST_RESULTS
    from concourse.bass_utils import run_bass_kernel_spmd

    x = np.ascontiguousarray(np.asarray(x, dtype=np.float32))
    Wr = np.asarray(Wr, dtype=np.float32)
    br = np.asarray(br, dtype=np.float32)
    W1 = np.ascontiguousarray(np.asarray(W1, dtype=np.float32))
    b1 = np.ascontiguousarray(np.asarray(b1, dtype=np.float32))
    W2 = np.ascontiguousarray(np.asarray(W2, dtype=np.float32))
    b2 = np.ascontiguousarray(np.asarray(b2, dtype=np.float32))

    B, S, H = x.shape
    ntok = B * S
    xf = x.reshape(ntok, H)

    topw, topi = _route_host(x, Wr, br)
    topw = topw.reshape(ntok, TOP_K)
    topi = topi.reshape(ntok, TOP_K)

    # per-expert token index lists + combine weights
    idx = []
    wgt = []
    for e in range(NUM_EXPERTS):
        mask = (topi == e)
        tok = np.nonzero(mask.any(axis=1))[0]
        w = (topw * mask).sum(axis=1)[tok].astype(np.float32)
        idx.append(tok)
        wgt.append(w)
    counts = np.array([len(t) for t in idx])

    blk = int(os.environ.get("MOE_BLK", "512"))
    C = max(_round_up(int(counts.max()), 2), 512)

    key = (C, blk)
    if key not in _PROGRAM_CACHE:
        _PROGRAM_CACHE[key] = _build_program(C, blk)
    nc = _PROGRAM_CACHE[key]

    in_maps = []
    for e in range(NUM_EXPERTS):
        xTe = np.zeros((H, C), dtype=np.float32)
        xTe[:, :counts[e]] = xf[idx[e]].T
        scle = np.zeros((C,), dtype=np.float32)
        scle[:counts[e]] = wgt[e]
        # pack weights into the kernel's SBUF tile layout:
        # w1p[p, g, c, fg] = W1[e][c*128+p, g*FPER*128+fg]
        W1G, W2G = 16, 8
        HC, FC = H // 128, 2 * H // 128
        FPER, CPER = FC // W1G, FC // W2G
        w1p = np.ascontiguousarray(
            W1[e].reshape(HC, 128, W1G, FPER * 128).transpose(1, 2, 0, 3))
        # w2p[p, g, cc, h] = W2[e][(g*CPER+cc)*128+p, h]
        w2p = np.ascontiguousarray(
            W2[e].reshape(W2G, CPER, 128, H).transpose(2, 0, 1, 3))
        in_maps.append({
            "xT": xTe,
            "w1p": w1p,
            "b1": np.ascontiguousarray(b1[e]),
            "w2p": w2p,
            "b2": np.ascontiguousarray(b2[e]),
            "scl": scle,
        })

    trace = os.environ.get("MOE_TRACE", "0") == "1"
    res = run_bass_kernel_spmd(
        nc, in_maps, core_ids=list(range(NCORES)), trace=trace)
    LAST_EXEC_NS = res.exec_time_ns
    LAST_RESULTS = res

    out = np.zeros((ntok, H), dtype=np.float32)
    for e in range(NUM_EXPERTS):
        ye = res.results[e]["yT"][:, :counts[e]].T  # [cnt, H]
        out[idx[e]] += ye
    return out.reshape(B, S, H)
